# revision 13
# baseline (speedup 1.0000x reference)
"""Trainium2 Bass kernel for nn_MultiHeadSelfAttention_11158325035343.

GQA multi-head self-attention (B=4, T=2048, E=2048, H=16, HKV=8, HD=128)
with XPos rotary embedding and causal softmax.

Sharding: 8 cores = 4 batches x 2 head-groups. Each core computes, for its
batch b and head-group g (8 q heads, 4 kv heads):
  QT/KT = W.T @ x.T   ([head_dim, T] per head, head_dim on partitions)
  V     = x @ W_v     ([T, head_dim] per kv head)
  XPos rope applied via two host-precomputed fused tables + half-swap
  scoresT[j, i] per (head, i-chunk, j-tile), exp without max subtraction
  (scores are bounded: XPos decay keeps them small), causal mask via 4
  precomputed diagonal tiles, softmax denominator via ones-matmul on PE,
  attnT = V.T-contraction with probs as moving operand, normalized by the
  PE-broadcast reciprocal denominator
  partial out = attnT.T @ W_o rows-for-this-group
Host sums the two group partials per batch.
"""

import sys
import types

sys.path.insert(0, "/opt/trn_rl_repo")

import numpy as np
import ml_dtypes

BF16 = ml_dtypes.bfloat16

# ---------------------------------------------------------------------------
# NTFF profile hook injection (missing antenv.axon_hooks in this image).
# Needed only when trace=True; harmless otherwise.
# ---------------------------------------------------------------------------
def _ensure_axon_hooks():
    if "antenv.axon_hooks" in sys.modules:
        return
    try:
        import antenv
        mod = types.ModuleType("antenv.axon_hooks")
        holder = {"hook": None}
        mod.set_axon_ntff_profile_hook = lambda h: holder.__setitem__("hook", h)
        mod.get_axon_ntff_profile_hook = lambda: holder["hook"]
        sys.modules["antenv.axon_hooks"] = mod
        antenv.axon_hooks = mod
        from trn_agent_boot.trn_boot import _ntff_profile_via_ctypes
        mod.set_axon_ntff_profile_hook(
            _ntff_profile_via_ctypes("/opt/axon/libaxon_pjrt.so")
        )
    except Exception:
        pass


_ensure_axon_hooks()

import concourse.bass as bass
import concourse.bacc as bacc
import concourse.mybir as mybir
import concourse.tile as tile
from concourse.bass_utils import run_bass_kernel_spmd

# Problem constants (hardcoded per spec).
B, T, E = 4, 2048, 2048
H, HKV, HD = 16, 8, 128
THETA, SCALE_BASE = 10000.0, 512.0
G = 2                   # head groups (cores per batch)
HL = H // G             # 8 local q heads
KVL = HKV // G          # 4 local kv heads
REP = H // HKV          # GQA repeat
CH = 512                # i-chunk / matmul free dim
NE = E // 128           # 16 contraction tiles
NF = HL + KVL           # 12 projection f-tiles (8 Q + 4 K)
HALFT = T // 2          # token half for phase-1 SBUF staging
NJT = T // 128          # 16 j tiles
NCH = T // CH           # 4 i chunks
INV_SQRT_D = 1.0 / float(np.sqrt(np.float32(HD)))

F32 = mybir.dt.float32
F16 = mybir.dt.float16
BF = mybir.dt.bfloat16

_COMPILED = None


def _build_nc():
    nc = bacc.Bacc("TRN2", target_bir_lowering=False, debug=False, num_devices=8)

    xt_d = nc.dram_tensor("xt", [E, T], BF, kind="ExternalInput")
    wqk_d = nc.dram_tensor("wqk", [NF, 128, NE, 128], BF, kind="ExternalInput")
    wv_d = nc.dram_tensor("wv", [E, KVL * HD], BF, kind="ExternalInput")
    wo_d = nc.dram_tensor("wo", [HL * HD, E], BF, kind="ExternalInput")
    aq_d = nc.dram_tensor("aq", [HD, T], BF, kind="ExternalInput")
    bq_d = nc.dram_tensor("bq", [HD, T], BF, kind="ExternalInput")
    ak_d = nc.dram_tensor("ak", [HD, T], BF, kind="ExternalInput")
    bk_d = nc.dram_tensor("bk", [HD, T], BF, kind="ExternalInput")
    mk_d = nc.dram_tensor("mk", [4, 128, CH], BF, kind="ExternalInput")
    id_d = nc.dram_tensor("ident", [128, 128], BF, kind="ExternalInput")
    ones_mat_d = nc.dram_tensor("ones_mat", [128, 128], BF, kind="ExternalInput")
    ones_row_d = nc.dram_tensor("ones_row", [1, 128], F16, kind="ExternalInput")
    out_d = nc.dram_tensor("out_p", [T, E], F32, kind="ExternalOutput")

    with tile.TileContext(nc) as tc:
        with (
            tc.tile_pool(name="xtwo", bufs=16) as pool_xt,      # xt halves, then wo
            tc.tile_pool(name="qk", bufs=NF) as pool_qk,        # rope'd QT/KT bf16
            tc.tile_pool(name="v", bufs=NJT) as pool_v,         # V bf16
            tc.tile_pool(name="at", bufs=HL) as pool_at,        # attnT bf16
            tc.tile_pool(name="tab", bufs=4) as pool_tab,       # rope tables
            tc.tile_pool(name="wv", bufs=NE) as pool_wv,        # resident W_v
            tc.tile_pool(name="w", bufs=2) as pool_w,           # streamed W_q/W_k
            tc.tile_pool(name="tmp", bufs=2) as pool_tmp,       # rope temp
            tc.tile_pool(name="p", bufs=4) as pool_p,           # exp probs bf16
            tc.tile_pool(name="bc", bufs=2) as pool_bc,         # denom broadcast
            tc.tile_pool(name="o", bufs=2) as pool_o,           # out staging
            tc.tile_pool(name="sm", bufs=1) as pool_sm,         # small constants
            tc.tile_pool(name="dv", bufs=2) as pool_dv,         # recip denominators
            tc.tile_pool(name="dvr", bufs=1) as pool_dvr,       # dinv row layout
            tc.tile_pool(name="ps", bufs=2, space=bass.MemorySpace.PSUM) as pool_ps,
        ):
            def load_constants():
                aq_t = pool_tab.tile([HD, T], BF, tag="tab", name="aq_t")
                nc.sync.dma_start(aq_t[:], aq_d[:])
                bq_t = pool_tab.tile([HD, T], BF, tag="tab", name="bq_t")
                nc.sync.dma_start(bq_t[:], bq_d[:])
                ak_t = pool_tab.tile([HD, T], BF, tag="tab", name="ak_t")
                nc.sync.dma_start(ak_t[:], ak_d[:])
                bk_t = pool_tab.tile([HD, T], BF, tag="tab", name="bk_t")
                nc.sync.dma_start(bk_t[:], bk_d[:])
                mask_t = []
                for dd in range(4):
                    mt = pool_tab.tile([128, CH], BF, tag="mask", name=f"mask{dd}")
                    nc.sync.dma_start(mt[:], mk_d[dd])
                    mask_t.append(mt)
                ident_t = pool_sm.tile([128, 128], BF, tag="ident", name="ident_t")
                nc.sync.dma_start(ident_t[:], id_d[:])
                ones_mat = pool_sm.tile([128, 128], BF, tag="oc", name="ones_mat")
                nc.sync.dma_start(ones_mat[:], ones_mat_d[:])
                ones_row = pool_sm.tile([1, 128], F16, tag="orow", name="ones_row")
                nc.sync.dma_start(ones_row[:], ones_row_d[:])
                wv_t = []
                for e in range(NE):
                    w = pool_wv.tile([128, KVL * HD], BF, tag="wv", name=f"wv{e}")
                    nc.sync.dma_start(w[:], wv_d[e * 128:(e + 1) * 128, :])
                    wv_t.append(w)
                return aq_t, bq_t, ak_t, bk_t, mask_t, ident_t, ones_mat, ones_row, wv_t

            # ---- persistent activation tensors ----
            qk_t = [pool_qk.tile([128, T], BF, tag="qk", name=f"qk{i}") for i in range(NF)]
            v_t = [pool_v.tile([128, KVL * HD], BF, tag="v", name=f"v{i}") for i in range(NJT)]
            at_t = [pool_at.tile([128, T], BF, tag="at", name=f"at{i}") for i in range(HL)]

            # ================= Phase 1: QKV projections + rope =============
            consts = None
            for half in range(2):
                hs = half * HALFT
                xt_t = []
                for e in range(NE):
                    xx = pool_xt.tile([128, HALFT], BF, tag="xtwo")
                    nc.sync.dma_start(
                        xx[:], xt_d[e * 128:(e + 1) * 128, hs:hs + HALFT]
                    )
                    xt_t.append(xx)
                if consts is None:
                    consts = load_constants()
                    (aq_t, bq_t, ak_t, bk_t, mask_t, ident_t, ones_mat,
                     ones_row, wv_t) = consts

                for f in range(NF):
                    # host-prearranged W column block, contiguous per partition
                    w_t = pool_w.tile([128, NE, 128], BF, tag="w")
                    nc.sync.dma_start(w_t[:], wqk_d[f])
                    for c in range(HALFT // CH):
                        ps = pool_ps.tile([128, CH], F32, tag="psacc", bufs=2)
                        for e in range(NE):
                            nc.tensor.matmul(
                                ps[:],
                                w_t[:, e, :],
                                xt_t[e][:, c * CH:(c + 1) * CH],
                                start=(e == 0),
                                stop=(e == NE - 1),
                            )
                        nc.vector.tensor_copy(
                            qk_t[f][:, hs + c * CH: hs + (c + 1) * CH], ps[:]
                        )
                    # rope over this token half
                    A_t, B_t = (aq_t, bq_t) if f < HL else (ak_t, bk_t)
                    q = qk_t[f]
                    sl = slice(hs, hs + HALFT)
                    qs = pool_tmp.tile([128, HALFT], BF, tag="qs")
                    nc.sync.dma_start(qs[0:64, :], q[64:128, sl])
                    nc.sync.dma_start(qs[64:128, :], q[0:64, sl])
                    nc.vector.tensor_mul(qs[:, :], qs[:, :], B_t[:, sl])
                    nc.vector.tensor_mul(q[:, sl], q[:, sl], A_t[:, sl])
                    nc.vector.tensor_add(q[:, sl], q[:, sl], qs[:])

                for tt in range(NJT // 2):
                    tglob = half * (NJT // 2) + tt
                    psv = pool_ps.tile([128, KVL * HD], F32, tag="psacc", bufs=2)
                    for e in range(NE):
                        nc.tensor.matmul(
                            psv[:],
                            xt_t[e][:, tt * 128:(tt + 1) * 128],
                            wv_t[e][:],
                            start=(e == 0),
                            stop=(e == NE - 1),
                        )
                    nc.vector.tensor_copy(v_t[tglob][:], psv[:])

            # W_o loads (into xt's slots, freed after phase 1): 16 x [128,1024]
            wo_t = []
            for fb in range(HL):
                row = []
                for hh in range(2):
                    w = pool_xt.tile([128, HALFT], BF, tag="xtwo")
                    nc.sync.dma_start(
                        w[:],
                        wo_d[fb * 128:(fb + 1) * 128, hh * HALFT:(hh + 1) * HALFT],
                    )
                    row.append(w)
                wo_t.append(row)

            # ================= Phase 2: attention ==========================
            F32R = mybir.dt.float32r
            for hl in range(HL):
                kf = HL + hl // REP
                kvc = (hl // REP) * HD
                # denominators for this head, laid out [128, 4] per chunk
                # (i = c*512 + p*4 + f) so the reciprocal runs on 128 lanes
                d128 = pool_dv.tile([128, NCH * 4], F32, tag="d128")
                pend = []   # deferred AV/den matmuls of the previous j-block
                for c in range(NCH):
                    njt = (c + 1) * (CH // 128)
                    acc = pool_ps.tile([128, CH], F32, tag="psacc", bufs=2)
                    den = pool_ps.tile([128, CH], F32, tag="psden", bufs=2)
                    for j0 in range(0, njt, 2):
                        # two j-tiles share one 2-bank PSUM tile so a single
                        # wide exp amortizes the ACT per-inst overhead
                        s2 = pool_ps.tile([128, 2, CH], F32, tag="ps", bufs=2)
                        for u in range(2):
                            jt = j0 + u
                            d = jt - (njt - 4)
                            nc.tensor.matmul(
                                s2[:, u, :],
                                qk_t[kf][:, jt * 128:(jt + 1) * 128],
                                qk_t[hl][:, c * CH:(c + 1) * CH],
                                start=True,
                                stop=(d < 0),
                            )
                            if d >= 0:
                                # add -1e9 to masked (j > i) positions, on-PE
                                nc.tensor.matmul(
                                    s2[:, u, :], ident_t[:], mask_t[d][:],
                                    start=False, stop=True,
                                )
                        p2 = pool_p.tile([128, 2, CH], BF, tag="p")
                        nc.scalar.activation(
                            p2[:], s2[:], mybir.ActivationFunctionType.Exp,
                            scale=INV_SQRT_D,
                        )
                        # emit the PREVIOUS block's AV/den so the PE has
                        # independent work while this block's exp runs
                        for fn in pend:
                            fn()
                        pend = []
                        for u in range(2):
                            jt = j0 + u
                            def av_den(jt=jt, p2=p2, u=u, acc=acc, den=den,
                                       njt=njt, kvc=kvc):
                                nc.tensor.matmul(
                                    acc[:],
                                    v_t[jt][:, kvc:kvc + HD],
                                    p2[:, u, :],
                                    start=(jt == 0),
                                    stop=(jt == njt - 1),
                                )
                                nc.tensor.matmul(
                                    den[:],
                                    ones_mat[:],
                                    p2[:, u, :],
                                    start=(jt == 0),
                                    stop=(jt == njt - 1),
                                )
                            pend.append(av_den)
                    # close out this chunk before normalization
                    for fn in pend:
                        fn()
                    pend = []
                    # unnormalized attnT out; free PSUM quickly
                    nc.vector.tensor_copy(
                        at_t[hl][:, c * CH:(c + 1) * CH], acc[:]
                    )
                    den_sb = pool_dv.tile([1, CH], F32, tag="densb")
                    nc.vector.tensor_scalar_mul(den_sb[:], den[0:1, :], 1.0 / 4096.0)
                    nc.sync.dma_start(d128[:, c * 4:(c + 1) * 4], den_sb[:])
                # ---- per-head normalization, off the j-loop critical path ----
                dr = pool_dv.tile([128, NCH * 4], F16, tag="dr")
                with nc.allow_low_precision(reason="dinv broadcast in fp16; 4096/den keeps it in normal range"):
                    nc.vector.reciprocal(dr[:], d128[:])
                dinv_row = pool_dvr.tile([1, T], F16, tag="dvrow")
                for c in range(NCH):
                    nc.sync.dma_start(
                        dinv_row[0:1, c * CH:(c + 1) * CH], dr[:, c * 4:(c + 1) * 4]
                    )
                for c in range(NCH):
                    bc_ps = pool_ps.tile([128, CH], F32, tag="psden", bufs=2)
                    nc.tensor.matmul(
                        bc_ps[:],
                        ones_row[:],
                        dinv_row[0:1, c * CH:(c + 1) * CH],
                        start=True,
                        stop=True,
                    )
                    bc_sb = pool_bc.tile([128, CH], F32, tag="bc")
                    nc.vector.tensor_copy(bc_sb[:], bc_ps[:])
                    nc.vector.tensor_mul(
                        at_t[hl][:, c * CH:(c + 1) * CH],
                        at_t[hl][:, c * CH:(c + 1) * CH],
                        bc_sb[:],
                    )

            # ================= Phase 3: output projection ==================
            for it in range(T // 128):
                for ec in range(E // CH):
                    po = pool_ps.tile([128, CH], F32, tag="psacc", bufs=2)
                    for fb in range(HL):
                        wsrc = wo_t[fb][ec // 2]
                        wof = (ec % 2) * CH
                        nc.tensor.matmul(
                            po[:],
                            at_t[fb][:, it * 128:(it + 1) * 128],
                            wsrc[:, wof:wof + CH],
                            start=(fb == 0),
                            stop=(fb == HL - 1),
                        )
                    os_t = pool_o.tile([128, CH], F32, tag="o")
                    nc.vector.tensor_copy(os_t[:], po[:])
                    nc.sync.dma_start(
                        out_d[it * 128:(it + 1) * 128, ec * CH:(ec + 1) * CH],
                        os_t[:],
                    )

    nc.compile()
    return nc


def _get_compiled():
    global _COMPILED
    if _COMPILED is None:
        _COMPILED = _build_nc()
    return _COMPILED


def _host_tables():
    half = np.arange(0, HD, 2, dtype=np.float64)
    inv_freq = 1.0 / (THETA ** (half / HD))
    t_idx = np.arange(T, dtype=np.float64)
    freqs = np.outer(t_idx, inv_freq)
    emb = np.concatenate([freqs, freqs], axis=-1)
    cos, sin = np.cos(emb), np.sin(emb)
    scale_vec = (half + 0.4 * HD) / (1.4 * HD)
    power = (t_idx - T // 2) / SCALE_BASE
    scale = scale_vec[None, :] ** power[:, None]
    scale = np.concatenate([scale, scale], axis=-1)
    sgn = np.where(np.arange(HD) < HD // 2, -1.0, 1.0)
    aq = (scale * cos).T
    bq = sgn[:, None] * (scale * sin).T
    ak = (cos / scale).T
    bk = sgn[:, None] * (sin / scale).T

    masks = np.zeros((4, 128, CH), np.float32)
    dj = np.arange(128)[:, None]
    di = np.arange(CH)[None, :]
    for k in range(4):
        masks[k] = np.where(128 * k + dj > di, -1e9, 0.0)
    return (
        aq.astype(BF16), bq.astype(BF16), ak.astype(BF16), bk.astype(BF16),
        masks.astype(BF16),
    )



def _arrange_wqk(wq, wk):
    # [E, F] -> per 128-wide f-block: [128(part=e%128), NE(e//128), 128(f)]
    w = np.concatenate([wq, wk], axis=1)          # [E, NF*128]
    nf = w.shape[1] // 128
    w = w.reshape(NE, 128, nf, 128)               # [n, p, f_blk, fc]
    w = w.transpose(2, 1, 0, 3)                   # [f_blk, p, n, fc]
    return np.ascontiguousarray(w).astype(BF16)


def _make_in_maps(x, W_q, W_k, W_v, W_o):
    aq, bq, ak, bk, masks = _host_tables()
    ones_mat = np.ones((128, 128), BF16)
    ones_row = np.ones((1, 128), np.float16)
    xts = [np.ascontiguousarray(x[b].T).astype(BF16) for b in range(B)]
    in_maps = []
    for core in range(8):
        b, g = core // G, core % G
        in_maps.append({
            "xt": xts[b],
            "wqk": _arrange_wqk(W_q[:, g * HL * HD:(g + 1) * HL * HD],
                                W_k[:, g * KVL * HD:(g + 1) * KVL * HD]),
            "wv": np.ascontiguousarray(W_v[:, g * KVL * HD:(g + 1) * KVL * HD]).astype(BF16),
            "wo": (np.ascontiguousarray(W_o[g * HL * HD:(g + 1) * HL * HD, :]) / 4096.0).astype(BF16),
            "aq": aq, "bq": bq, "ak": ak, "bk": bk,
            "mk": masks,
            "ident": np.eye(128, dtype=BF16),
            "ones_mat": ones_mat,
            "ones_row": ones_row,
        })
    return in_maps


def _run(x, W_q, W_k, W_v, W_o, trace=False):
    nc = _get_compiled()
    in_maps = _make_in_maps(x, W_q, W_k, W_v, W_o)
    res = run_bass_kernel_spmd(nc, in_maps, list(range(8)), trace=trace)
    out = np.empty((B, T, E), np.float32)
    for b in range(B):
        out[b] = res.results[2 * b]["out_p"] + res.results[2 * b + 1]["out_p"]
    return out, res.exec_time_ns


def kernel(x, W_q, W_k, W_v, W_o):
    out, _ = _run(
        np.asarray(x), np.asarray(W_q), np.asarray(W_k),
        np.asarray(W_v), np.asarray(W_o),
    )
    return out


# revision 14
# speedup vs baseline: 1.0757x; 1.0757x over previous
"""Trainium2 Bass kernel for nn_MultiHeadSelfAttention_11158325035343.

GQA multi-head self-attention (B=4, T=2048, E=2048, H=16, HKV=8, HD=128)
with XPos rotary embedding and causal softmax.

Sharding: 8 cores = 4 batches x 2 head-groups. Each core computes, for its
batch b and head-group g (8 q heads, 4 kv heads):
  QT/KT = W.T @ x.T   ([head_dim, T] per head, head_dim on partitions)
  V     = x @ W_v     ([T, head_dim] per kv head)
  XPos rope applied via two host-precomputed fused tables + half-swap
  scoresT[j, i] per (head, i-chunk, j-tile), exp without max subtraction
  (scores are bounded: XPos decay keeps them small), causal mask via 4
  precomputed diagonal tiles, softmax denominator via ones-matmul on PE,
  attnT = V.T-contraction with probs as moving operand, normalized by the
  PE-broadcast reciprocal denominator
  partial out = attnT.T @ W_o rows-for-this-group
Host sums the two group partials per batch.
"""

import sys
import types

sys.path.insert(0, "/opt/trn_rl_repo")

import numpy as np
import ml_dtypes

BF16 = ml_dtypes.bfloat16

# ---------------------------------------------------------------------------
# NTFF profile hook injection (missing antenv.axon_hooks in this image).
# Needed only when trace=True; harmless otherwise.
# ---------------------------------------------------------------------------
def _ensure_axon_hooks():
    if "antenv.axon_hooks" in sys.modules:
        return
    try:
        import antenv
        mod = types.ModuleType("antenv.axon_hooks")
        holder = {"hook": None}
        mod.set_axon_ntff_profile_hook = lambda h: holder.__setitem__("hook", h)
        mod.get_axon_ntff_profile_hook = lambda: holder["hook"]
        sys.modules["antenv.axon_hooks"] = mod
        antenv.axon_hooks = mod
        from trn_agent_boot.trn_boot import _ntff_profile_via_ctypes
        mod.set_axon_ntff_profile_hook(
            _ntff_profile_via_ctypes("/opt/axon/libaxon_pjrt.so")
        )
    except Exception:
        pass


_ensure_axon_hooks()

import concourse.bass as bass
import concourse.bacc as bacc
import concourse.mybir as mybir
import concourse.tile as tile
from concourse.bass_utils import run_bass_kernel_spmd

# Problem constants (hardcoded per spec).
B, T, E = 4, 2048, 2048
H, HKV, HD = 16, 8, 128
THETA, SCALE_BASE = 10000.0, 512.0
G = 2                   # head groups (cores per batch)
HL = H // G             # 8 local q heads
KVL = HKV // G          # 4 local kv heads
REP = H // HKV          # GQA repeat
CH = 512                # i-chunk / matmul free dim
NE = E // 128           # 16 contraction tiles
NF = HL + KVL           # 12 projection f-tiles (8 Q + 4 K)
HALFT = T // 2          # token half for phase-1 SBUF staging
NJT = T // 128          # 16 j tiles
NCH = T // CH           # 4 i chunks
INV_SQRT_D = 1.0 / float(np.sqrt(np.float32(HD)))

F32 = mybir.dt.float32
F16 = mybir.dt.float16
BF = mybir.dt.bfloat16

_COMPILED = None


def _build_nc():
    nc = bacc.Bacc("TRN2", target_bir_lowering=False, debug=False, num_devices=8)

    xt_d = nc.dram_tensor("xt", [E, T], BF, kind="ExternalInput")
    wqk_d = nc.dram_tensor("wqk", [NF, 128, NE, 128], BF, kind="ExternalInput")
    wv_d = nc.dram_tensor("wv", [E, KVL * HD], BF, kind="ExternalInput")
    wo_d = nc.dram_tensor("wo", [HL * HD, E], BF, kind="ExternalInput")
    aq_d = nc.dram_tensor("aq", [HD, T], BF, kind="ExternalInput")
    bq_d = nc.dram_tensor("bq", [HD, T], BF, kind="ExternalInput")
    ak_d = nc.dram_tensor("ak", [HD, T], BF, kind="ExternalInput")
    bk_d = nc.dram_tensor("bk", [HD, T], BF, kind="ExternalInput")
    mk_d = nc.dram_tensor("mk", [4, 128, CH], BF, kind="ExternalInput")
    id_d = nc.dram_tensor("ident", [128, 128], BF, kind="ExternalInput")
    ones_mat_d = nc.dram_tensor("ones_mat", [128, 128], BF, kind="ExternalInput")
    out_d = nc.dram_tensor("out_p", [T, E], F32, kind="ExternalOutput")

    with tile.TileContext(nc) as tc:
        with (
            tc.tile_pool(name="xtwo", bufs=16) as pool_xt,      # xt halves, then wo
            tc.tile_pool(name="qk", bufs=NF) as pool_qk,        # rope'd QT/KT bf16
            tc.tile_pool(name="v", bufs=NJT) as pool_v,         # V bf16
            tc.tile_pool(name="at", bufs=HL) as pool_at,        # attnT bf16
            tc.tile_pool(name="tab", bufs=4) as pool_tab,       # rope tables
            tc.tile_pool(name="wv", bufs=NE) as pool_wv,        # resident W_v
            tc.tile_pool(name="w", bufs=2) as pool_w,           # streamed W_q/W_k
            tc.tile_pool(name="tmp", bufs=2) as pool_tmp,       # rope temp
            tc.tile_pool(name="p", bufs=4) as pool_p,           # exp probs bf16
            tc.tile_pool(name="o", bufs=2) as pool_o,           # out staging
            tc.tile_pool(name="sm", bufs=1) as pool_sm,         # small constants
            tc.tile_pool(name="dv", bufs=2) as pool_dv,         # recip denominators
            tc.tile_pool(name="dvr", bufs=2, space="DRAM") as pool_dvr,  # dinv DRAM bounce
            tc.tile_pool(name="bch", bufs=2) as pool_bch,       # dinv bcast per head
            tc.tile_pool(name="ps", bufs=2, space=bass.MemorySpace.PSUM) as pool_ps,
        ):
            def load_constants():
                aq_t = pool_tab.tile([HD, T], BF, tag="tab", name="aq_t")
                nc.sync.dma_start(aq_t[:], aq_d[:])
                bq_t = pool_tab.tile([HD, T], BF, tag="tab", name="bq_t")
                nc.sync.dma_start(bq_t[:], bq_d[:])
                ak_t = pool_tab.tile([HD, T], BF, tag="tab", name="ak_t")
                nc.sync.dma_start(ak_t[:], ak_d[:])
                bk_t = pool_tab.tile([HD, T], BF, tag="tab", name="bk_t")
                nc.sync.dma_start(bk_t[:], bk_d[:])
                mask_t = []
                for dd in range(4):
                    mt = pool_tab.tile([128, CH], BF, tag="mask", name=f"mask{dd}")
                    nc.sync.dma_start(mt[:], mk_d[dd])
                    mask_t.append(mt)
                ident_t = pool_sm.tile([128, 128], BF, tag="ident", name="ident_t")
                nc.sync.dma_start(ident_t[:], id_d[:])
                ones_mat = pool_sm.tile([128, 128], BF, tag="oc", name="ones_mat")
                nc.sync.dma_start(ones_mat[:], ones_mat_d[:])
                wv_t = []
                for e in range(NE):
                    w = pool_wv.tile([128, KVL * HD], BF, tag="wv", name=f"wv{e}")
                    nc.sync.dma_start(w[:], wv_d[e * 128:(e + 1) * 128, :])
                    wv_t.append(w)
                return aq_t, bq_t, ak_t, bk_t, mask_t, ident_t, ones_mat, wv_t

            # ---- persistent activation tensors ----
            qk_t = [pool_qk.tile([128, T], BF, tag="qk", name=f"qk{i}") for i in range(NF)]
            v_t = [pool_v.tile([128, KVL * HD], BF, tag="v", name=f"v{i}") for i in range(NJT)]
            at_t = [pool_at.tile([128, T], BF, tag="at", name=f"at{i}") for i in range(HL)]

            # ================= Phase 1: QKV projections + rope =============
            consts = None
            for half in range(2):
                hs = half * HALFT
                w_next = pool_w.tile([128, NE, 128], BF, tag="w", name="w_pre")
                nc.sync.dma_start(w_next[:], wqk_d[0])
                xt_t = []
                for e in range(NE):
                    xx = pool_xt.tile([128, HALFT], BF, tag="xtwo")
                    nc.sync.dma_start(
                        xx[:], xt_d[e * 128:(e + 1) * 128, hs:hs + HALFT]
                    )
                    xt_t.append(xx)
                if consts is None:
                    consts = load_constants()
                    (aq_t, bq_t, ak_t, bk_t, mask_t, ident_t, ones_mat,
                     wv_t) = consts

                for f in range(NF):
                    # host-prearranged W column block, contiguous per partition
                    w_t = w_next
                    if f + 1 < NF:
                        w_next = pool_w.tile([128, NE, 128], BF, tag="w",
                                             name=f"w_pre{f + 1}")
                        nc.sync.dma_start(w_next[:], wqk_d[f + 1])
                    for c in range(HALFT // CH):
                        ps = pool_ps.tile([128, CH], F32, tag="psacc", bufs=2)
                        for e in range(NE):
                            nc.tensor.matmul(
                                ps[:],
                                w_t[:, e, :],
                                xt_t[e][:, c * CH:(c + 1) * CH],
                                start=(e == 0),
                                stop=(e == NE - 1),
                            )
                        nc.vector.tensor_copy(
                            qk_t[f][:, hs + c * CH: hs + (c + 1) * CH], ps[:]
                        )
                    # rope over this token half
                    A_t, B_t = (aq_t, bq_t) if f < HL else (ak_t, bk_t)
                    q = qk_t[f]
                    sl = slice(hs, hs + HALFT)
                    qs = pool_tmp.tile([128, HALFT], BF, tag="qs")
                    nc.sync.dma_start(qs[0:64, :], q[64:128, sl])
                    nc.sync.dma_start(qs[64:128, :], q[0:64, sl])
                    nc.vector.tensor_mul(qs[:, :], qs[:, :], B_t[:, sl])
                    nc.vector.tensor_mul(q[:, sl], q[:, sl], A_t[:, sl])
                    nc.vector.tensor_add(q[:, sl], q[:, sl], qs[:])

                for tt in range(NJT // 2):
                    tglob = half * (NJT // 2) + tt
                    psv = pool_ps.tile([128, KVL * HD], F32, tag="psacc", bufs=2)
                    for e in range(NE):
                        nc.tensor.matmul(
                            psv[:],
                            xt_t[e][:, tt * 128:(tt + 1) * 128],
                            wv_t[e][:],
                            start=(e == 0),
                            stop=(e == NE - 1),
                        )
                    nc.vector.tensor_copy(v_t[tglob][:], psv[:])

            # W_o loads (into xt's slots, freed after phase 1): 16 x [128,1024]
            wo_t = []
            for fb in range(HL):
                row = []
                for hh in range(2):
                    w = pool_xt.tile([128, HALFT], BF, tag="xtwo")
                    nc.sync.dma_start(
                        w[:],
                        wo_d[fb * 128:(fb + 1) * 128, hh * HALFT:(hh + 1) * HALFT],
                    )
                    row.append(w)
                wo_t.append(row)

            # ================= Phase 2: attention ==========================
            F32R = mybir.dt.float32r
            for hl in range(HL):
                kf = HL + hl // REP
                kvc = (hl // REP) * HD
                # denominators for this head, laid out [128, 4] per chunk
                # (i = c*512 + p*4 + f) so the reciprocal runs on 128 lanes
                d128 = pool_dv.tile([128, NCH * 4], F32, tag="d128")
                pend = []   # deferred AV/den matmuls of the previous j-block
                for c in range(NCH):
                    njt = (c + 1) * (CH // 128)
                    acc = pool_ps.tile([128, CH], F32, tag="psacc", bufs=2)
                    den = pool_ps.tile([128, CH], F32, tag="psden", bufs=2)
                    for j0 in range(0, njt, 2):
                        # two j-tiles share one 2-bank PSUM tile so a single
                        # wide exp amortizes the ACT per-inst overhead
                        s2 = pool_ps.tile([128, 2, CH], F32, tag="ps", bufs=2)
                        for u in range(2):
                            jt = j0 + u
                            d = jt - (njt - 4)
                            nc.tensor.matmul(
                                s2[:, u, :],
                                qk_t[kf][:, jt * 128:(jt + 1) * 128],
                                qk_t[hl][:, c * CH:(c + 1) * CH],
                                start=True,
                                stop=(d < 0),
                            )
                            if d >= 0:
                                # add -1e9 to masked (j > i) positions, on-PE
                                nc.tensor.matmul(
                                    s2[:, u, :], ident_t[:], mask_t[d][:],
                                    start=False, stop=True,
                                )
                        p2 = pool_p.tile([128, 2, CH], BF, tag="p")
                        nc.scalar.activation(
                            p2[:], s2[:], mybir.ActivationFunctionType.Exp,
                            scale=INV_SQRT_D,
                        )
                        # emit the PREVIOUS block's AV/den so the PE has
                        # independent work while this block's exp runs
                        for fn in pend:
                            fn()
                        pend = []
                        for u in range(2):
                            jt = j0 + u
                            def av_den(jt=jt, p2=p2, u=u, acc=acc, den=den,
                                       njt=njt, kvc=kvc):
                                nc.tensor.matmul(
                                    acc[:],
                                    v_t[jt][:, kvc:kvc + HD],
                                    p2[:, u, :],
                                    start=(jt == 0),
                                    stop=(jt == njt - 1),
                                )
                                nc.tensor.matmul(
                                    den[:],
                                    ones_mat[:],
                                    p2[:, u, :],
                                    start=(jt == 0),
                                    stop=(jt == njt - 1),
                                )
                            pend.append(av_den)
                    # close out this chunk before normalization
                    for fn in pend:
                        fn()
                    pend = []
                    # unnormalized attnT out; free PSUM quickly
                    nc.vector.tensor_copy(
                        at_t[hl][:, c * CH:(c + 1) * CH], acc[:]
                    )
                    den_sb = pool_dv.tile([1, CH], F32, tag="densb")
                    nc.vector.tensor_scalar_mul(den_sb[:], den[0:1, :], 1.0 / 4096.0)
                    nc.sync.dma_start(d128[:, c * 4:(c + 1) * 4], den_sb[:])
                # ---- per-head normalization, off the j-loop critical path ----
                dr = pool_dv.tile([128, NCH * 4], F16, tag="dr")
                with nc.allow_low_precision(reason="dinv broadcast in fp16; 4096/den keeps it in normal range"):
                    nc.vector.reciprocal(dr[:], d128[:])
                dinv_dram = pool_dvr.tile([1, T], F16, tag="dvrow")
                for c in range(NCH):
                    nc.sync.dma_start(
                        dinv_dram[0:1, c * CH:(c + 1) * CH], dr[:, c * 4:(c + 1) * 4]
                    )
                # broadcast to all partitions via a 0-stride DRAM read
                bch = pool_bch.tile([128, T], F16, tag="bch")
                nc.sync.dma_start(bch[:], dinv_dram[:].to_broadcast((128, T)))
                for c in range(NCH):
                    nc.vector.tensor_mul(
                        at_t[hl][:, c * CH:(c + 1) * CH],
                        at_t[hl][:, c * CH:(c + 1) * CH],
                        bch[:, c * CH:(c + 1) * CH],
                    )

            # ================= Phase 3: output projection ==================
            for it in range(T // 128):
                for ec in range(E // CH):
                    po = pool_ps.tile([128, CH], F32, tag="psacc", bufs=2)
                    for fb in range(HL):
                        wsrc = wo_t[fb][ec // 2]
                        wof = (ec % 2) * CH
                        nc.tensor.matmul(
                            po[:],
                            at_t[fb][:, it * 128:(it + 1) * 128],
                            wsrc[:, wof:wof + CH],
                            start=(fb == 0),
                            stop=(fb == HL - 1),
                        )
                    os_t = pool_o.tile([128, CH], F32, tag="o")
                    nc.vector.tensor_copy(os_t[:], po[:])
                    nc.sync.dma_start(
                        out_d[it * 128:(it + 1) * 128, ec * CH:(ec + 1) * CH],
                        os_t[:],
                    )

    nc.compile()
    return nc


def _get_compiled():
    global _COMPILED
    if _COMPILED is None:
        _COMPILED = _build_nc()
    return _COMPILED


def _host_tables():
    half = np.arange(0, HD, 2, dtype=np.float64)
    inv_freq = 1.0 / (THETA ** (half / HD))
    t_idx = np.arange(T, dtype=np.float64)
    freqs = np.outer(t_idx, inv_freq)
    emb = np.concatenate([freqs, freqs], axis=-1)
    cos, sin = np.cos(emb), np.sin(emb)
    scale_vec = (half + 0.4 * HD) / (1.4 * HD)
    power = (t_idx - T // 2) / SCALE_BASE
    scale = scale_vec[None, :] ** power[:, None]
    scale = np.concatenate([scale, scale], axis=-1)
    sgn = np.where(np.arange(HD) < HD // 2, -1.0, 1.0)
    aq = (scale * cos).T
    bq = sgn[:, None] * (scale * sin).T
    ak = (cos / scale).T
    bk = sgn[:, None] * (sin / scale).T

    masks = np.zeros((4, 128, CH), np.float32)
    dj = np.arange(128)[:, None]
    di = np.arange(CH)[None, :]
    for k in range(4):
        masks[k] = np.where(128 * k + dj > di, -1e9, 0.0)
    return (
        aq.astype(BF16), bq.astype(BF16), ak.astype(BF16), bk.astype(BF16),
        masks.astype(BF16),
    )



def _arrange_wqk(wq, wk):
    # [E, F] -> per 128-wide f-block: [128(part=e%128), NE(e//128), 128(f)]
    w = np.concatenate([wq, wk], axis=1)          # [E, NF*128]
    nf = w.shape[1] // 128
    w = w.reshape(NE, 128, nf, 128)               # [n, p, f_blk, fc]
    w = w.transpose(2, 1, 0, 3)                   # [f_blk, p, n, fc]
    return np.ascontiguousarray(w).astype(BF16)


def _make_in_maps(x, W_q, W_k, W_v, W_o):
    aq, bq, ak, bk, masks = _host_tables()
    ones_mat = np.ones((128, 128), BF16)
    xts = [np.ascontiguousarray(x[b].T).astype(BF16) for b in range(B)]
    in_maps = []
    for core in range(8):
        b, g = core // G, core % G
        in_maps.append({
            "xt": xts[b],
            "wqk": _arrange_wqk(W_q[:, g * HL * HD:(g + 1) * HL * HD],
                                W_k[:, g * KVL * HD:(g + 1) * KVL * HD]),
            "wv": np.ascontiguousarray(W_v[:, g * KVL * HD:(g + 1) * KVL * HD]).astype(BF16),
            "wo": (np.ascontiguousarray(W_o[g * HL * HD:(g + 1) * HL * HD, :]) / 4096.0).astype(BF16),
            "aq": aq, "bq": bq, "ak": ak, "bk": bk,
            "mk": masks,
            "ident": np.eye(128, dtype=BF16),
            "ones_mat": ones_mat,
        })
    return in_maps


def _run(x, W_q, W_k, W_v, W_o, trace=False):
    nc = _get_compiled()
    in_maps = _make_in_maps(x, W_q, W_k, W_v, W_o)
    res = run_bass_kernel_spmd(nc, in_maps, list(range(8)), trace=trace)
    out = np.empty((B, T, E), np.float32)
    for b in range(B):
        out[b] = res.results[2 * b]["out_p"] + res.results[2 * b + 1]["out_p"]
    return out, res.exec_time_ns


def kernel(x, W_q, W_k, W_v, W_o):
    out, _ = _run(
        np.asarray(x), np.asarray(W_q), np.asarray(W_k),
        np.asarray(W_v), np.asarray(W_o),
    )
    return out


# revision 15
# speedup vs baseline: 1.0841x; 1.0078x over previous
"""Trainium2 Bass kernel for nn_MultiHeadSelfAttention_11158325035343.

GQA multi-head self-attention (B=4, T=2048, E=2048, H=16, HKV=8, HD=128)
with XPos rotary embedding and causal softmax.

Sharding: 8 cores = 4 batches x 2 head-groups. Each core computes, for its
batch b and head-group g (8 q heads, 4 kv heads):
  QT/KT = W.T @ x.T   ([head_dim, T] per head, head_dim on partitions)
  V     = x @ W_v     ([T, head_dim] per kv head)
  XPos rope applied via two host-precomputed fused tables + half-swap
  scoresT[j, i] per (head, i-chunk, j-tile), exp without max subtraction
  (scores are bounded: XPos decay keeps them small), causal mask via 4
  precomputed diagonal tiles, softmax denominator via ones-matmul on PE,
  attnT = V.T-contraction with probs as moving operand, normalized by the
  PE-broadcast reciprocal denominator
  partial out = attnT.T @ W_o rows-for-this-group
Host sums the two group partials per batch.
"""

import sys
import types

sys.path.insert(0, "/opt/trn_rl_repo")

import numpy as np
import ml_dtypes

BF16 = ml_dtypes.bfloat16

# ---------------------------------------------------------------------------
# NTFF profile hook injection (missing antenv.axon_hooks in this image).
# Needed only when trace=True; harmless otherwise.
# ---------------------------------------------------------------------------
def _ensure_axon_hooks():
    if "antenv.axon_hooks" in sys.modules:
        return
    try:
        import antenv
        mod = types.ModuleType("antenv.axon_hooks")
        holder = {"hook": None}
        mod.set_axon_ntff_profile_hook = lambda h: holder.__setitem__("hook", h)
        mod.get_axon_ntff_profile_hook = lambda: holder["hook"]
        sys.modules["antenv.axon_hooks"] = mod
        antenv.axon_hooks = mod
        from trn_agent_boot.trn_boot import _ntff_profile_via_ctypes
        mod.set_axon_ntff_profile_hook(
            _ntff_profile_via_ctypes("/opt/axon/libaxon_pjrt.so")
        )
    except Exception:
        pass


_ensure_axon_hooks()

import concourse.bass as bass
import concourse.bacc as bacc
import concourse.mybir as mybir
import concourse.tile as tile
from concourse.bass_utils import run_bass_kernel_spmd

# Problem constants (hardcoded per spec).
B, T, E = 4, 2048, 2048
H, HKV, HD = 16, 8, 128
THETA, SCALE_BASE = 10000.0, 512.0
G = 2                   # head groups (cores per batch)
HL = H // G             # 8 local q heads
KVL = HKV // G          # 4 local kv heads
REP = H // HKV          # GQA repeat
CH = 512                # i-chunk / matmul free dim
NE = E // 128           # 16 contraction tiles
NF = HL + KVL           # 12 projection f-tiles (8 Q + 4 K)
HALFT = T // 2          # token half for phase-1 SBUF staging
NJT = T // 128          # 16 j tiles
NCH = T // CH           # 4 i chunks
INV_SQRT_D = 1.0 / float(np.sqrt(np.float32(HD)))

F32 = mybir.dt.float32
F16 = mybir.dt.float16
BF = mybir.dt.bfloat16

_COMPILED = None


def _build_nc():
    nc = bacc.Bacc("TRN2", target_bir_lowering=False, debug=False, num_devices=8)

    xt_d = nc.dram_tensor("xt", [E, T], BF, kind="ExternalInput")
    wqk_d = nc.dram_tensor("wqk", [NF, 128, NE, 128], BF, kind="ExternalInput")
    wv_d = nc.dram_tensor("wv", [E, KVL * HD], BF, kind="ExternalInput")
    wo_d = nc.dram_tensor("wo", [HL * HD, E], BF, kind="ExternalInput")
    aq_d = nc.dram_tensor("aq", [HD, T], BF, kind="ExternalInput")
    bq_d = nc.dram_tensor("bq", [HD, T], BF, kind="ExternalInput")
    ak_d = nc.dram_tensor("ak", [HD, T], BF, kind="ExternalInput")
    bk_d = nc.dram_tensor("bk", [HD, T], BF, kind="ExternalInput")
    mk_d = nc.dram_tensor("mk", [4, 128, CH], BF, kind="ExternalInput")
    id_d = nc.dram_tensor("ident", [128, 128], BF, kind="ExternalInput")
    ones_mat_d = nc.dram_tensor("ones_mat", [128, 128], BF, kind="ExternalInput")
    out_d = nc.dram_tensor("out_p", [T, E], F32, kind="ExternalOutput")

    with tile.TileContext(nc) as tc:
        with (
            tc.tile_pool(name="xtwo", bufs=16) as pool_xt,      # xt halves, then wo
            tc.tile_pool(name="qk", bufs=NF) as pool_qk,        # rope'd QT/KT bf16
            tc.tile_pool(name="v", bufs=NJT) as pool_v,         # V bf16
            tc.tile_pool(name="at", bufs=HL) as pool_at,        # attnT bf16
            tc.tile_pool(name="tab", bufs=4) as pool_tab,       # rope tables
            tc.tile_pool(name="wv", bufs=NE) as pool_wv,        # resident W_v
            tc.tile_pool(name="w", bufs=3) as pool_w,           # streamed W_q/W_k
            tc.tile_pool(name="tmp", bufs=2) as pool_tmp,       # rope temp
            tc.tile_pool(name="p", bufs=4) as pool_p,           # exp probs bf16
            tc.tile_pool(name="o", bufs=2) as pool_o,           # out staging
            tc.tile_pool(name="sm", bufs=1) as pool_sm,         # small constants
            tc.tile_pool(name="dv", bufs=2) as pool_dv,         # recip denominators
            tc.tile_pool(name="dvr", bufs=2, space="DRAM") as pool_dvr,  # dinv DRAM bounce
            tc.tile_pool(name="bch", bufs=2) as pool_bch,       # dinv bcast per head
            tc.tile_pool(name="ps", bufs=2, space=bass.MemorySpace.PSUM) as pool_ps,
        ):
            def load_constants():
                aq_t = pool_tab.tile([HD, T], BF, tag="tab", name="aq_t")
                nc.sync.dma_start(aq_t[:], aq_d[:])
                bq_t = pool_tab.tile([HD, T], BF, tag="tab", name="bq_t")
                nc.sync.dma_start(bq_t[:], bq_d[:])
                ak_t = pool_tab.tile([HD, T], BF, tag="tab", name="ak_t")
                nc.sync.dma_start(ak_t[:], ak_d[:])
                bk_t = pool_tab.tile([HD, T], BF, tag="tab", name="bk_t")
                nc.sync.dma_start(bk_t[:], bk_d[:])
                mask_t = []
                for dd in range(4):
                    mt = pool_tab.tile([128, CH], BF, tag="mask", name=f"mask{dd}")
                    nc.sync.dma_start(mt[:], mk_d[dd])
                    mask_t.append(mt)
                ident_t = pool_sm.tile([128, 128], BF, tag="ident", name="ident_t")
                nc.sync.dma_start(ident_t[:], id_d[:])
                ones_mat = pool_sm.tile([128, 128], BF, tag="oc", name="ones_mat")
                nc.sync.dma_start(ones_mat[:], ones_mat_d[:])
                wv_t = []
                for e in range(NE):
                    w = pool_wv.tile([128, KVL * HD], BF, tag="wv", name=f"wv{e}")
                    nc.sync.dma_start(w[:], wv_d[e * 128:(e + 1) * 128, :])
                    wv_t.append(w)
                return aq_t, bq_t, ak_t, bk_t, mask_t, ident_t, ones_mat, wv_t

            # ---- persistent activation tensors ----
            qk_t = [pool_qk.tile([128, T], BF, tag="qk", name=f"qk{i}") for i in range(NF)]
            v_t = [pool_v.tile([128, KVL * HD], BF, tag="v", name=f"v{i}") for i in range(NJT)]
            at_t = [pool_at.tile([128, T], BF, tag="at", name=f"at{i}") for i in range(HL)]

            # ================= Phase 1: QKV projections + rope =============
            consts = None
            for half in range(2):
                hs = half * HALFT
                wq_pre = []

                def w_prefetch(f, half=half):
                    w = pool_w.tile([128, NE, 128], BF, tag="w",
                                    name=f"w_pre{half}_{f}")
                    nc.sync.dma_start(w[:], wqk_d[f])
                    wq_pre.append(w)

                w_prefetch(0)
                xt_t = []
                for e in range(NE):
                    xx = pool_xt.tile([128, HALFT], BF, tag="xtwo")
                    nc.sync.dma_start(
                        xx[:], xt_d[e * 128:(e + 1) * 128, hs:hs + HALFT]
                    )
                    xt_t.append(xx)
                w_prefetch(1)
                w_prefetch(2)
                if consts is None:
                    consts = load_constants()
                    (aq_t, bq_t, ak_t, bk_t, mask_t, ident_t, ones_mat,
                     wv_t) = consts

                for f in range(NF):
                    # host-prearranged W column block, contiguous per partition
                    w_t = wq_pre.pop(0)
                    if f + 3 < NF:
                        w_prefetch(f + 3)
                    for c in range(HALFT // CH):
                        ps = pool_ps.tile([128, CH], F32, tag="psacc", bufs=2)
                        for e in range(NE):
                            nc.tensor.matmul(
                                ps[:],
                                w_t[:, e, :],
                                xt_t[e][:, c * CH:(c + 1) * CH],
                                start=(e == 0),
                                stop=(e == NE - 1),
                            )
                        nc.vector.tensor_copy(
                            qk_t[f][:, hs + c * CH: hs + (c + 1) * CH], ps[:]
                        )
                    # rope over this token half
                    A_t, B_t = (aq_t, bq_t) if f < HL else (ak_t, bk_t)
                    q = qk_t[f]
                    sl = slice(hs, hs + HALFT)
                    qs = pool_tmp.tile([128, HALFT], BF, tag="qs")
                    nc.sync.dma_start(qs[0:64, :], q[64:128, sl])
                    nc.sync.dma_start(qs[64:128, :], q[0:64, sl])
                    nc.vector.tensor_mul(qs[:, :], qs[:, :], B_t[:, sl])
                    nc.vector.tensor_mul(q[:, sl], q[:, sl], A_t[:, sl])
                    nc.vector.tensor_add(q[:, sl], q[:, sl], qs[:])

                for tt in range(NJT // 2):
                    tglob = half * (NJT // 2) + tt
                    psv = pool_ps.tile([128, KVL * HD], F32, tag="psacc", bufs=2)
                    for e in range(NE):
                        nc.tensor.matmul(
                            psv[:],
                            xt_t[e][:, tt * 128:(tt + 1) * 128],
                            wv_t[e][:],
                            start=(e == 0),
                            stop=(e == NE - 1),
                        )
                    nc.vector.tensor_copy(v_t[tglob][:], psv[:])

            # W_o loads (into xt's slots, freed after phase 1): 16 x [128,1024]
            wo_t = []
            for fb in range(HL):
                row = []
                for hh in range(2):
                    w = pool_xt.tile([128, HALFT], BF, tag="xtwo")
                    nc.gpsimd.dma_start(
                        w[:],
                        wo_d[fb * 128:(fb + 1) * 128, hh * HALFT:(hh + 1) * HALFT],
                    )
                    row.append(w)
                wo_t.append(row)

            # ================= Phase 2: attention ==========================
            F32R = mybir.dt.float32r
            for hl in range(HL):
                kf = HL + hl // REP
                kvc = (hl // REP) * HD
                # denominators for this head, laid out [128, 4] per chunk
                # (i = c*512 + p*4 + f) so the reciprocal runs on 128 lanes
                d128 = pool_dv.tile([128, NCH * 4], F32, tag="d128")
                pend = []   # deferred AV/den matmuls of the previous j-block
                for c in range(NCH):
                    njt = (c + 1) * (CH // 128)
                    acc = pool_ps.tile([128, CH], F32, tag="psacc", bufs=2)
                    den = pool_ps.tile([128, CH], F32, tag="psden", bufs=2)
                    for j0 in range(0, njt, 2):
                        # two j-tiles share one 2-bank PSUM tile so a single
                        # wide exp amortizes the ACT per-inst overhead
                        s2 = pool_ps.tile([128, 2, CH], F32, tag="ps", bufs=2)
                        for u in range(2):
                            jt = j0 + u
                            d = jt - (njt - 4)
                            nc.tensor.matmul(
                                s2[:, u, :],
                                qk_t[kf][:, jt * 128:(jt + 1) * 128],
                                qk_t[hl][:, c * CH:(c + 1) * CH],
                                start=True,
                                stop=(d < 0),
                            )
                            if d >= 0:
                                # add -1e9 to masked (j > i) positions, on-PE
                                nc.tensor.matmul(
                                    s2[:, u, :], ident_t[:], mask_t[d][:],
                                    start=False, stop=True,
                                )
                        p2 = pool_p.tile([128, 2, CH], BF, tag="p")
                        nc.scalar.activation(
                            p2[:], s2[:], mybir.ActivationFunctionType.Exp,
                            scale=INV_SQRT_D,
                        )
                        # emit the PREVIOUS block's AV/den so the PE has
                        # independent work while this block's exp runs
                        for fn in pend:
                            fn()
                        pend = []
                        for u in range(2):
                            jt = j0 + u
                            def av_den(jt=jt, p2=p2, u=u, acc=acc, den=den,
                                       njt=njt, kvc=kvc):
                                nc.tensor.matmul(
                                    acc[:],
                                    v_t[jt][:, kvc:kvc + HD],
                                    p2[:, u, :],
                                    start=(jt == 0),
                                    stop=(jt == njt - 1),
                                )
                                nc.tensor.matmul(
                                    den[:],
                                    ones_mat[:],
                                    p2[:, u, :],
                                    start=(jt == 0),
                                    stop=(jt == njt - 1),
                                )
                            pend.append(av_den)
                    # close out this chunk before normalization
                    for fn in pend:
                        fn()
                    pend = []
                    # unnormalized attnT out; free PSUM quickly
                    nc.vector.tensor_copy(
                        at_t[hl][:, c * CH:(c + 1) * CH], acc[:]
                    )
                    den_sb = pool_dv.tile([1, CH], F32, tag="densb")
                    nc.vector.tensor_scalar_mul(den_sb[:], den[0:1, :], 1.0 / 4096.0)
                    nc.sync.dma_start(d128[:, c * 4:(c + 1) * 4], den_sb[:])
                # ---- per-head normalization, off the j-loop critical path ----
                dr = pool_dv.tile([128, NCH * 4], F16, tag="dr")
                with nc.allow_low_precision(reason="dinv broadcast in fp16; 4096/den keeps it in normal range"):
                    nc.vector.reciprocal(dr[:], d128[:])
                dinv_dram = pool_dvr.tile([1, T], F16, tag="dvrow")
                for c in range(NCH):
                    nc.sync.dma_start(
                        dinv_dram[0:1, c * CH:(c + 1) * CH], dr[:, c * 4:(c + 1) * 4]
                    )
                # broadcast to all partitions via a 0-stride DRAM read
                bch = pool_bch.tile([128, T], F16, tag="bch")
                nc.sync.dma_start(bch[:], dinv_dram[:].to_broadcast((128, T)))
                for c in range(NCH):
                    nc.vector.tensor_mul(
                        at_t[hl][:, c * CH:(c + 1) * CH],
                        at_t[hl][:, c * CH:(c + 1) * CH],
                        bch[:, c * CH:(c + 1) * CH],
                    )

            # ================= Phase 3: output projection ==================
            for it in range(T // 128):
                for ec in range(E // CH):
                    po = pool_ps.tile([128, CH], F32, tag="psacc", bufs=2)
                    for fb in range(HL):
                        wsrc = wo_t[fb][ec // 2]
                        wof = (ec % 2) * CH
                        nc.tensor.matmul(
                            po[:],
                            at_t[fb][:, it * 128:(it + 1) * 128],
                            wsrc[:, wof:wof + CH],
                            start=(fb == 0),
                            stop=(fb == HL - 1),
                        )
                    os_t = pool_o.tile([128, CH], F32, tag="o")
                    nc.vector.tensor_copy(os_t[:], po[:])
                    nc.sync.dma_start(
                        out_d[it * 128:(it + 1) * 128, ec * CH:(ec + 1) * CH],
                        os_t[:],
                    )

    nc.compile()
    return nc


def _get_compiled():
    global _COMPILED
    if _COMPILED is None:
        _COMPILED = _build_nc()
    return _COMPILED


def _host_tables():
    half = np.arange(0, HD, 2, dtype=np.float64)
    inv_freq = 1.0 / (THETA ** (half / HD))
    t_idx = np.arange(T, dtype=np.float64)
    freqs = np.outer(t_idx, inv_freq)
    emb = np.concatenate([freqs, freqs], axis=-1)
    cos, sin = np.cos(emb), np.sin(emb)
    scale_vec = (half + 0.4 * HD) / (1.4 * HD)
    power = (t_idx - T // 2) / SCALE_BASE
    scale = scale_vec[None, :] ** power[:, None]
    scale = np.concatenate([scale, scale], axis=-1)
    sgn = np.where(np.arange(HD) < HD // 2, -1.0, 1.0)
    aq = (scale * cos).T
    bq = sgn[:, None] * (scale * sin).T
    ak = (cos / scale).T
    bk = sgn[:, None] * (sin / scale).T

    masks = np.zeros((4, 128, CH), np.float32)
    dj = np.arange(128)[:, None]
    di = np.arange(CH)[None, :]
    for k in range(4):
        masks[k] = np.where(128 * k + dj > di, -1e9, 0.0)
    return (
        aq.astype(BF16), bq.astype(BF16), ak.astype(BF16), bk.astype(BF16),
        masks.astype(BF16),
    )



def _arrange_wqk(wq, wk):
    # [E, F] -> per 128-wide f-block: [128(part=e%128), NE(e//128), 128(f)]
    w = np.concatenate([wq, wk], axis=1)          # [E, NF*128]
    nf = w.shape[1] // 128
    w = w.reshape(NE, 128, nf, 128)               # [n, p, f_blk, fc]
    w = w.transpose(2, 1, 0, 3)                   # [f_blk, p, n, fc]
    return np.ascontiguousarray(w).astype(BF16)


def _make_in_maps(x, W_q, W_k, W_v, W_o):
    aq, bq, ak, bk, masks = _host_tables()
    ones_mat = np.ones((128, 128), BF16)
    xts = [np.ascontiguousarray(x[b].T).astype(BF16) for b in range(B)]
    in_maps = []
    for core in range(8):
        b, g = core // G, core % G
        in_maps.append({
            "xt": xts[b],
            "wqk": _arrange_wqk(W_q[:, g * HL * HD:(g + 1) * HL * HD],
                                W_k[:, g * KVL * HD:(g + 1) * KVL * HD]),
            "wv": np.ascontiguousarray(W_v[:, g * KVL * HD:(g + 1) * KVL * HD]).astype(BF16),
            "wo": (np.ascontiguousarray(W_o[g * HL * HD:(g + 1) * HL * HD, :]) / 4096.0).astype(BF16),
            "aq": aq, "bq": bq, "ak": ak, "bk": bk,
            "mk": masks,
            "ident": np.eye(128, dtype=BF16),
            "ones_mat": ones_mat,
        })
    return in_maps


def _run(x, W_q, W_k, W_v, W_o, trace=False):
    nc = _get_compiled()
    in_maps = _make_in_maps(x, W_q, W_k, W_v, W_o)
    res = run_bass_kernel_spmd(nc, in_maps, list(range(8)), trace=trace)
    out = np.empty((B, T, E), np.float32)
    for b in range(B):
        out[b] = res.results[2 * b]["out_p"] + res.results[2 * b + 1]["out_p"]
    return out, res.exec_time_ns


def kernel(x, W_q, W_k, W_v, W_o):
    out, _ = _run(
        np.asarray(x), np.asarray(W_q), np.asarray(W_k),
        np.asarray(W_v), np.asarray(W_o),
    )
    return out
